# revision 8
# baseline (speedup 1.0000x reference)
"""Trainium2 Bass kernel for a CrossTransformerBlock (point-transformer style
vector attention over K=32 neighbors of N=8192 keypoints).

Sharding: data-parallel over the N keypoint axis across 8 NeuronCores; the
small MLP weights are replicated (with some host-side weight folding).

Math (per keypoint n, neighbor k):
    q   = feats_q @ Wq
    kat = feats_kv @ Wk ;  v = feats_kv @ Wv
    pos = relu(rel @ d1 + d1_b) @ d2 + d2_b ,  rel = xyz_q - xyz_kv
    h   = q - kat + pos
    s   = relu(h @ g1 + g1_b) @ g2 + g2_b
    attn= softmax_k(s)
    res = (sum_k attn * (v + pos)) @ out_w + out_b

Device-side reformulation (everything kept feature-on-partition, "^T world"):
    scores_pre = feats_kv @ (-(Wk@g1)) + relu(rel@d1+d1_b) @ (d2@g1)
                 + bcast_n(feats_q @ (Wq@g1)) ;  + bias (d2_b@g1 + g1_b) in relu
    values     = feats_kv @ Wv + relu(rel@d1+d1_b) @ d2       (d2_b folded into
                 the output bias: sum_k attn = 1)
    e          = exp(relu(scores_pre)@g2 + g2_b)   (softmax without max-sub:
                 scores are O(1) by construction)
    res        = (sum_k e * values) / (sum_k e)
    out        = res @ out_w + (out_b + d2_b@out_w)
Broadcasts over k of per-n terms are done with indicator-matrix matmuls on the
tensor engine.  Matmul inputs use float32r (~tf32) — rel-err ~1e-4.
"""

import numpy as np

import concourse.bass as bass
import concourse.mybir as mybir
import concourse.tile as tile
from concourse import bacc
from concourse.bass_utils import run_bass_kernel_spmd

F32 = mybir.dt.float32
F32R = mybir.dt.float32r
AF = mybir.ActivationFunctionType

N, K = 8192, 32
DIM, DIMH, DOUT = 256, 128, 512
NCORES = 8
NPC = N // NCORES  # keypoints per core


def _build_nc(npc: int, stage: int = 99):
    """Build the per-core Bass program. npc = keypoints handled by the core.

    stage (debug): 0 = preamble only, N>=1 = preamble + N blocks (+tail if all
    blocks emitted)."""
    nblk = npc * K // 512          # 512 (n,k)-rows per block
    ntile_n = npc // 128           # 128-keypoint tiles

    nc = bacc.Bacc("TRN2", debug=False, num_devices=NCORES)

    # ---- dram I/O ----
    fkv = nc.dram_tensor("fkv", [npc * K, DIM], F32, kind="ExternalInput")
    xkv = nc.dram_tensor("xkv", [npc * K, 3], F32, kind="ExternalInput")
    fq = nc.dram_tensor("fq", [npc, DIM], F32, kind="ExternalInput")
    xq = nc.dram_tensor("xq", [npc, 3], F32, kind="ExternalInput")
    w_kg1n = nc.dram_tensor("w_kg1n", [DIM, DIM], F32, kind="ExternalInput")
    w_v = nc.dram_tensor("w_v", [DIM, DIM], F32, kind="ExternalInput")
    w_qg1 = nc.dram_tensor("w_qg1", [DIM, DIM], F32, kind="ExternalInput")
    w_d2g1 = nc.dram_tensor("w_d2g1", [DIMH, DIM], F32, kind="ExternalInput")
    w_d2 = nc.dram_tensor("w_d2", [DIMH, DIM], F32, kind="ExternalInput")
    w_g2 = nc.dram_tensor("w_g2", [DIM, DIM], F32, kind="ExternalInput")
    w_d1 = nc.dram_tensor("w_d1", [3, DIMH], F32, kind="ExternalInput")
    w_d1n = nc.dram_tensor("w_d1n", [3, DIMH], F32, kind="ExternalInput")
    w_out = nc.dram_tensor("w_out", [DIM, DOUT], F32, kind="ExternalInput")
    b_sc = nc.dram_tensor("b_sc", [DIM], F32, kind="ExternalInput")
    b_g2 = nc.dram_tensor("b_g2", [DIM], F32, kind="ExternalInput")
    b_d1 = nc.dram_tensor("b_d1", [DIMH], F32, kind="ExternalInput")
    b_out2 = nc.dram_tensor("b_out2", [1, DOUT], F32, kind="ExternalInput")
    bmat = nc.dram_tensor("bmat", [128, 8, 512], F32, kind="ExternalInput")
    ident_in = nc.dram_tensor("ident_in", [128, 128], F32, kind="ExternalInput")
    ones_in = nc.dram_tensor("ones_in", [1, 128], F32, kind="ExternalInput")
    res_out = nc.dram_tensor("res_out", [npc, DOUT], F32, kind="ExternalOutput")

    with tile.TileContext(nc) as tc:
        with (
            tc.tile_pool(name="const", bufs=1) as cp,
            tc.tile_pool(name="persist", bufs=1) as pp,
            tc.tile_pool(name="work", bufs=3) as wp,
            tc.tile_pool(name="ps", bufs=2, space="PSUM") as ps,
        ):
            # ---------- constants ----------
            wkg1n_t = cp.tile([128, 2, DIM], F32R)
            nc.gpsimd.dma_start(out=wkg1n_t, in_=w_kg1n.ap().rearrange("(cc p) d -> p cc d", p=128))
            wv_t = cp.tile([128, 2, DIM], F32R)
            nc.gpsimd.dma_start(out=wv_t, in_=w_v.ap().rearrange("(cc p) d -> p cc d", p=128))
            wqg1_t = cp.tile([128, 2, DIM], F32R)
            nc.gpsimd.dma_start(out=wqg1_t, in_=w_qg1.ap().rearrange("(cc p) d -> p cc d", p=128))
            d2g1_t = cp.tile([128, DIM], F32R)
            nc.gpsimd.dma_start(out=d2g1_t, in_=w_d2g1.ap())
            d2_t = cp.tile([128, DIM], F32R)
            nc.gpsimd.dma_start(out=d2_t, in_=w_d2.ap())
            g2_t = cp.tile([128, 2, DIM], F32R)
            nc.gpsimd.dma_start(out=g2_t, in_=w_g2.ap().rearrange("(cc p) d -> p cc d", p=128))
            d1_t = cp.tile([3, DIMH], F32R)
            nc.gpsimd.dma_start(out=d1_t, in_=w_d1.ap())
            d1n_t = cp.tile([3, DIMH], F32R)
            nc.gpsimd.dma_start(out=d1n_t, in_=w_d1n.ap())
            outw_t = cp.tile([128, 2, DOUT], F32R)
            nc.gpsimd.dma_start(out=outw_t, in_=w_out.ap().rearrange("(j p) o -> p j o", p=128))
            bmat_t = cp.tile([128, 8, 512], F32R)
            nc.gpsimd.dma_start(out=bmat_t, in_=bmat.ap())
            outb2_t = cp.tile([1, DOUT], F32R)
            nc.gpsimd.dma_start(out=outb2_t, in_=b_out2.ap())
            ones_t = cp.tile([1, 128], F32R)
            nc.gpsimd.dma_start(out=ones_t, in_=ones_in.ap())
            ident_t = cp.tile([128, 128], F32)
            nc.sync.dma_start(out=ident_t, in_=ident_in.ap())
            bsc_t = cp.tile([128, 2], F32)
            nc.sync.dma_start(out=bsc_t, in_=b_sc.ap().rearrange("(j p) -> p j", p=128))
            bg2_t = cp.tile([128, 2], F32)
            nc.sync.dma_start(out=bg2_t, in_=b_g2.ap().rearrange("(j p) -> p j", p=128))
            bd1_t = cp.tile([128, 1], F32)
            nc.sync.dma_start(out=bd1_t, in_=b_d1.ap().rearrange("(j p) -> p j", p=128))

            # ---------- preamble: per-keypoint q terms ----------
            # feats_q^T (c on partitions) via PE transposes
            fqT = pp.tile([128, 2, npc], F32R)  # [c_in_chunk, cc, n]
            fq_r = fq.ap().rearrange("(t p) c -> t p c", p=128)
            for t in range(ntile_n):
                fq_nat = wp.tile([128, DIM], F32, tag="fq_nat", bufs=2)
                nc.sync.dma_start(out=fq_nat, in_=fq_r[t])
                pst = ps.tile([128, 512], F32, tag="T")
                for cc in range(2):
                    nc.tensor.transpose(
                        pst[:, cc * 128 : cc * 128 + 128],
                        fq_nat[:, cc * 128 : cc * 128 + 128],
                        ident_t,
                    )
                for cc in range(2):
                    nc.vector.tensor_copy(
                        fqT[:, cc, t * 128 : t * 128 + 128],
                        pst[:, cc * 128 : cc * 128 + 128],
                    )
            xqT = pp.tile([3, npc], F32R)
            xq_r = xq.ap().rearrange("(t p) c -> t p c", p=128)
            for t in range(ntile_n):
                xq_nat = wp.tile([128, 3], F32, tag="xq_nat", bufs=2)
                nc.sync.dma_start(out=xq_nat, in_=xq_r[t])
                psx = ps.tile([128, 512], F32, tag="X", bufs=1)
                nc.tensor.transpose(psx[0:3, 0:128], xq_nat, ident_t)
                nc.vector.tensor_copy(xqT[:, t * 128 : t * 128 + 128], psx[0:3, 0:128])

            # qg1_nat[n_part, t, d] = feats_q @ (Wq@g1);  qd1_nat = xyz_q @ d1
            qg1_nat = pp.tile([128, ntile_n, DIM], F32R)
            qd1_nat = pp.tile([128, ntile_n, DIMH], F32R)
            for t in range(ntile_n):
                psq = ps.tile([128, 512], F32, tag="S")
                for cc in range(2):
                    nc.tensor.matmul(
                        psq[:, 0:DIM],
                        fqT[:, cc, t * 128 : t * 128 + 128],
                        wqg1_t[:, cc, :],
                        start=(cc == 0),
                        stop=(cc == 1),
                    )
                nc.vector.tensor_copy(qg1_nat[:, t, :], psq[:, 0:DIM])
                psd = ps.tile([128, 512], F32, tag="V")
                nc.tensor.matmul(
                    psd[:, 0:DIMH],
                    xqT[:, t * 128 : t * 128 + 128],
                    d1_t,
                    start=True,
                    stop=True,
                )
                nc.vector.tensor_copy(qd1_nat[:, t, :], psd[:, 0:DIMH])

            # persistent accumulators for the softmax-weighted sums
            num_sb = [pp.tile([128, npc], F32, name=f"num{j}") for j in range(2)]
            den_sb = [pp.tile([128, npc], F32, name=f"den{j}") for j in range(2)]

            # ---------- main loop over 512-row (n,k) blocks ----------
            if stage == 0:
                # debug: dump qg1_nat to output and stop
                dump = wp.tile([128, DIM], F32, tag="osb")
                nc.vector.tensor_copy(dump, qg1_nat[:, 0, :].bitcast(F32))
                nc.sync.dma_start(out=res_out[0:128, 0:DIM], in_=dump)

            fkv_r = fkv.ap().rearrange("(b t p) c -> b p t c", p=128, t=4)
            xkv_r = xkv.ap().rearrange("(b t p) c -> b p t c", p=128, t=4)
            for b in range(min(nblk, stage)):
                t = b // 8
                u = b % 8

                feats_nat = wp.tile([128, 4, DIM], F32, tag="feats_nat")
                nc.sync.dma_start(out=feats_nat, in_=fkv_r[b])
                xyz_nat = wp.tile([128, 4, 3], F32, tag="xyz_nat")
                nc.sync.dma_start(out=xyz_nat, in_=xkv_r[b])

                # transpose feats/xyz chunks into ^T layout (c, nk)
                ftr = []
                for cc in range(2):
                    pst = ps.tile([128, 512], F32, tag="T")
                    for t4 in range(4):
                        nc.tensor.transpose(
                            pst[:, t4 * 128 : t4 * 128 + 128],
                            feats_nat[:, t4, cc * 128 : cc * 128 + 128],
                            ident_t,
                        )
                    f = wp.tile([128, 512], F32R, tag="ftr", bufs=4, name=f"ftr{cc}")
                    nc.vector.tensor_copy(f, pst)
                    ftr.append(f)
                psx = ps.tile([128, 512], F32, tag="X", bufs=1)
                for t4 in range(4):
                    nc.tensor.transpose(
                        psx[0:3, t4 * 128 : t4 * 128 + 128],
                        xyz_nat[:, t4, :],
                        ident_t,
                    )
                xtr = wp.tile([3, 512], F32R, tag="xtr", bufs=2)
                nc.vector.tensor_copy(xtr, psx[0:3, :])

                # pos-MLP first layer: (rel @ d1)^T  (+q bcast), relu
                psp = ps.tile([128, 512], F32, tag="P", bufs=1)
                nc.tensor.matmul(psp, d1n_t, xtr, start=True, stop=False)
                nc.tensor.matmul(
                    psp,
                    qd1_nat[:, t, :],
                    bmat_t[:, u, :],
                    start=False,
                    stop=True,
                )
                relu_pos = wp.tile([128, 512], F32R, tag="relu_pos", bufs=2)
                nc.scalar.activation(relu_pos, psp, AF.Relu, bias=bd1_t[:, 0:1])

                # scores_pre^T and values^T (two 128-d chunks each)
                psS = []
                for j in range(2):
                    pssj = ps.tile([128, 512], F32, tag="S", name=f"psS{j}")
                    for cc in range(2):
                        nc.tensor.matmul(
                            pssj,
                            wkg1n_t[:, cc, j * 128 : j * 128 + 128],
                            ftr[cc],
                            start=(cc == 0),
                            stop=False,
                        )
                    nc.tensor.matmul(
                        pssj,
                        qg1_nat[:, t, j * 128 : j * 128 + 128],
                        bmat_t[:, u, :],
                        start=False,
                        stop=False,
                    )
                    nc.tensor.matmul(
                        pssj,
                        d2g1_t[:, j * 128 : j * 128 + 128],
                        relu_pos,
                        start=False,
                        stop=True,
                    )
                    psS.append(pssj)
                psV = []
                for j in range(2):
                    psvj = ps.tile([128, 512], F32, tag="V", name=f"psV{j}")
                    for cc in range(2):
                        nc.tensor.matmul(
                            psvj,
                            wv_t[:, cc, j * 128 : j * 128 + 128],
                            ftr[cc],
                            start=(cc == 0),
                            stop=False,
                        )
                    nc.tensor.matmul(
                        psvj,
                        d2_t[:, j * 128 : j * 128 + 128],
                        relu_pos,
                        start=False,
                        stop=True,
                    )
                    psV.append(psvj)

                relu_sc = []
                for j in range(2):
                    r = wp.tile([128, 512], F32R, tag="relu_sc", bufs=4, name=f"relu_sc{j}")
                    nc.scalar.activation(r, psS[j], AF.Relu, bias=bsc_t[:, j : j + 1])
                    relu_sc.append(r)
                vsb = []
                for j in range(2):
                    v = wp.tile([128, 512], F32, tag="vsb", bufs=4, name=f"vsb{j}")
                    nc.scalar.activation(v, psV[j], AF.Copy)
                    vsb.append(v)

                # gamma second layer + exp
                for j in range(2):
                    psg = ps.tile([128, 512], F32, tag="S", name=f"psG{j}")
                    for cc in range(2):
                        nc.tensor.matmul(
                            psg,
                            g2_t[:, cc, j * 128 : j * 128 + 128],
                            relu_sc[cc],
                            start=(cc == 0),
                            stop=(cc == 1),
                        )
                    expt = wp.tile([128, 512], F32, tag="expt", bufs=4, name=f"expt{j}")
                    nc.scalar.activation(expt, psg, AF.Exp, bias=bg2_t[:, j : j + 1])
                    prod = wp.tile([128, 512], F32, tag="prod", bufs=4, name=f"prod{j}")
                    nc.gpsimd.tensor_mul(prod, expt, vsb[j])
                    nc.vector.tensor_reduce(
                        out=num_sb[j][:, 16 * b : 16 * b + 16],
                        in_=prod.rearrange("p (n k) -> p n k", k=K),
                        axis=mybir.AxisListType.X,
                        op=mybir.AluOpType.add,
                    )
                    nc.vector.tensor_reduce(
                        out=den_sb[j][:, 16 * b : 16 * b + 16],
                        in_=expt.rearrange("p (n k) -> p n k", k=K),
                        axis=mybir.AxisListType.X,
                        op=mybir.AluOpType.add,
                    )

            # ---------- tail: divide + output projection ----------
            if 0 < stage < nblk:
                # debug: dump partial num/den sums and stop
                dump2 = wp.tile([128, 512], F32, tag="osb")
                nc.vector.memset(dump2, 0.0)
                nc.vector.tensor_copy(dump2[:, 0 : 16 * stage], num_sb[0][:, 0 : 16 * stage])
                nc.vector.tensor_copy(dump2[:, 256 : 256 + 16 * stage], den_sb[0][:, 0 : 16 * stage])
                nc.sync.dma_start(out=res_out[0:128, 0:512], in_=dump2)
            if stage >= nblk:
                resT = []
                for j in range(2):
                    rc = pp.tile([128, npc], F32, name=f"recip{j}")
                    nc.vector.reciprocal(rc, den_sb[j])
                    rt = pp.tile([128, npc], F32R, name=f"resT{j}")
                    nc.vector.tensor_mul(rt, num_sb[j], rc)
                    resT.append(rt)
                for t in range(ntile_n):
                    pso = ps.tile([128, 512], F32, tag="V", name="psO")
                    for j in range(2):
                        nc.tensor.matmul(
                            pso,
                            resT[j][:, t * 128 : t * 128 + 128],
                            outw_t[:, j, :],
                            start=(j == 0),
                            stop=False,
                        )
                    nc.tensor.matmul(pso, ones_t, outb2_t, start=False, stop=True)
                    osb = wp.tile([128, DOUT], F32, tag="osb", bufs=2)
                    nc.scalar.activation(osb, pso, AF.Copy)
                    nc.sync.dma_start(out=res_out[t * 128 : t * 128 + 128, :], in_=osb)

    nc.finalize()
    return nc


_NC_CACHE: dict = {}


def _get_nc(npc: int):
    if npc not in _NC_CACHE:
        _NC_CACHE[npc] = _build_nc(npc)
    return _NC_CACHE[npc]


def _host_fold(Wq, Wk, Wv, d1_w, d1_b, d2_w, d2_b, g1_w, g1_b, g2_w, g2_b, out_w, out_b):
    wqg1 = (Wq @ g1_w).astype(np.float32)
    wkg1n = (-(Wk @ g1_w)).astype(np.float32)
    d2g1 = (d2_w @ g1_w).astype(np.float32)
    b_sc = (d2_b @ g1_w + g1_b).astype(np.float32)
    b_out2 = (out_b + d2_b @ out_w).astype(np.float32).reshape(1, DOUT)
    bmat = np.zeros((128, 8, 512), np.float32)
    for u in range(8):
        for v in range(16):
            bmat[16 * u + v, u, 32 * v : 32 * v + 32] = 1.0
    return dict(
        w_qg1=wqg1,
        w_kg1n=wkg1n,
        w_v=np.ascontiguousarray(Wv, np.float32),
        w_d2g1=d2g1,
        w_d2=np.ascontiguousarray(d2_w, np.float32),
        w_g2=np.ascontiguousarray(g2_w, np.float32),
        w_d1=np.ascontiguousarray(d1_w, np.float32),
        w_d1n=np.ascontiguousarray(-d1_w, np.float32),
        w_out=np.ascontiguousarray(out_w, np.float32),
        b_sc=b_sc,
        b_g2=np.ascontiguousarray(g2_b, np.float32),
        b_d1=np.ascontiguousarray(d1_b, np.float32),
        b_out2=b_out2,
        bmat=bmat,
        ident_in=np.eye(128, dtype=np.float32),
        ones_in=np.ones((1, 128), np.float32),
    )


def kernel(
    xyz_q,
    feats_q,
    xyz_kv,
    feats_kv,
    Wq,
    Wk,
    Wv,
    d1_w,
    d1_b,
    d2_w,
    d2_b,
    g1_w,
    g1_b,
    g2_w,
    g2_b,
    out_w,
    out_b,
):
    xyz_q = np.asarray(xyz_q, np.float32)
    feats_q = np.asarray(feats_q, np.float32)
    xyz_kv = np.asarray(xyz_kv, np.float32)
    feats_kv = np.asarray(feats_kv, np.float32)
    n = xyz_q.shape[0]
    npc = n // NCORES
    nc = _get_nc(npc)

    consts = _host_fold(
        np.asarray(Wq, np.float32), np.asarray(Wk, np.float32), np.asarray(Wv, np.float32),
        np.asarray(d1_w, np.float32), np.asarray(d1_b, np.float32),
        np.asarray(d2_w, np.float32), np.asarray(d2_b, np.float32),
        np.asarray(g1_w, np.float32), np.asarray(g1_b, np.float32),
        np.asarray(g2_w, np.float32), np.asarray(g2_b, np.float32),
        np.asarray(out_w, np.float32), np.asarray(out_b, np.float32),
    )

    in_maps = []
    for c in range(NCORES):
        lo, hi = c * npc, (c + 1) * npc
        m = dict(consts)
        m["fkv"] = np.ascontiguousarray(feats_kv[lo:hi].reshape(npc * K, DIM))
        m["xkv"] = np.ascontiguousarray(xyz_kv[lo:hi].reshape(npc * K, 3))
        m["fq"] = np.ascontiguousarray(feats_q[lo:hi])
        m["xq"] = np.ascontiguousarray(xyz_q[lo:hi])
        in_maps.append(m)

    res = run_bass_kernel_spmd(nc, in_maps, core_ids=list(range(NCORES)))
    out = np.concatenate([res.results[c]["res_out"] for c in range(NCORES)], axis=0)
    return (xyz_q, out)
